# revision 8
# baseline (speedup 1.0000x reference)
"""Trainium2 Bass kernel for BaselineFeedforwardNetwork recurrence.

Reference computation (per path, T=60 steps, serial in t):
    x_t = [features_t (8), delta_{t-1} (1)]            # (9,)
    h1  = relu(x_t @ W1 + b1)                          # (128,)
    h2  = relu(h1 @ W2 + b2)                           # (128,)
    d_t = h2 @ W3 + b3                                 # (1,)
Output: deltas (N, T).

Data-parallel over N=65536 paths across 8 NeuronCores (8192/core),
weights replicated, recurrence local per core.

Per-core dataflow (bf16 matmuls, fp32 PSUM):
- 16 path tiles of 512; activations transposed (feature/hidden dim on
  partitions, paths on free dim). Tile i = 4s+g -> pack s (0..3),
  row-group g (0..3).
- Features preloaded to SBUF in 15-step mega-chunks (1 DMA per chunk
  per row group) - dma_start issue costs ~1us of engine time, so DMA
  count per step must be ~1.
- mm1a: K=8 feature matmul, 4x row-tiled (groups at partitions 32g).
- mm1b: K=1 rank-1 delta matmul accumulated into the same psum bank,
  reading the PREVIOUS step's delta straight from SBUF at partition
  32g - no cross-partition delta move needed.
- mm2: full K=128 matmul per tile.
- mm3: block-diagonal W3 (column 32g of a (128,97) slice of W3sp), so
  the 4 tiles of a pack accumulate their deltas into ONE psum bank at
  rows {0,32,64,96} - the delta eviction is a single ACT op per pack,
  and the delta lands exactly where mm1b needs it.
- Evictions (PSUM->SBUF + bias + relu): split between ScalarE ACT and
  VectorE tensor_scalar to balance the two engines (the throughput
  bottleneck at ~712-743ns per (128,512) tile).
- One out-DMA per step writes the step's deltas to DRAM.
"""

import os
import sys

import numpy as np

for _p in ("/opt/trn_rl_repo", "/root/.axon_site/_ro/trn_rl_repo"):
    if _p not in sys.path and os.path.isdir(_p):
        sys.path.append(_p)

import ml_dtypes  # noqa: E402

N_FULL = 65536
T_FULL = 60
F = 8
HID = 128
N_CORES = 8
NSH = N_FULL // N_CORES  # 8192 paths per core

BF16 = ml_dtypes.bfloat16


def build_kernel(nsh=NSH, t_steps=T_FULL, num_cores=N_CORES, b3_value=0.0,
                 chunk_steps=15, pair_h1=False, pair_pd=False, merge_pd=False,
                 t_decl=None, repeats=1):
    """Builds the per-core Bass graph. Returns the compiled nc.

    t_decl: declared time extent of the DRAM tensors (defaults to t_steps).
    repeats: run the whole t_steps loop R times inside one NEFF (timing only;
    the recurrence restarts at each repeat). Both knobs exist so the timing
    harness can compare identical-I/O NEFFs whose device time differs by a
    known step count, making the slope independent of dispatch/transfer cost.
    """
    import concourse.bass as bass
    import concourse.tile as tile
    from concourse import bacc, mybir

    if t_decl is None:
        t_decl = t_steps
    bf = mybir.dt.bfloat16
    f32 = mybir.dt.float32
    NT = 512                       # path-tile width (one fp32 psum bank)
    ntiles = nsh // NT
    npacks = ntiles // 4           # pack = 4 row-tiled tiles
    xw = npacks * NT               # per-step free width
    TC = min(chunk_steps, t_steps)
    assert ntiles % 4 == 0

    nc = bacc.Bacc(
        "TRN2", target_bir_lowering=False, debug=False,
        num_devices=num_cores,
    )

    feat = nc.declare_dram_parameter("features", [4, F, t_decl, xw], bf, isOutput=False)
    w1p = nc.declare_dram_parameter("W1p", [128, HID], bf, isOutput=False)
    w1d = nc.declare_dram_parameter("W1d", [128, HID], bf, isOutput=False)
    w2 = nc.declare_dram_parameter("W2", [HID, HID], bf, isOutput=False)
    w3sp = nc.declare_dram_parameter("W3sp", [128, 8, 113], bf, isOutput=False)
    b1 = nc.declare_dram_parameter("b1", [128, 1], f32, isOutput=False)
    b2 = nc.declare_dram_parameter("b2", [128, 1], f32, isOutput=False)
    out = nc.declare_dram_parameter("out", [t_decl, 4, xw], bf, isOutput=True)

    Relu = mybir.ActivationFunctionType.Relu
    Copy = mybir.ActivationFunctionType.Copy
    add = mybir.AluOpType.add
    amax = mybir.AluOpType.max

    with tile.TileContext(nc) as tc:
        with (
            tc.tile_pool(name="consts", bufs=1) as cpool,
            tc.tile_pool(name="f", bufs=2) as fpool,
            tc.tile_pool(name="h1r", bufs=6) as h1pool,
            tc.tile_pool(name="h2r", bufs=4) as h2pool,
            tc.tile_pool(name="dst", bufs=2) as dpool,
            tc.tile_pool(name="ph1", bufs=2 if pair_h1 else 4, space="PSUM") as ph1pool,
            tc.tile_pool(name="ph2", bufs=2, space="PSUM") as ph2pool,
            tc.tile_pool(name="pd", bufs=1 if pair_pd else 2, space="PSUM") as pdpool,
        ):
            w1sb = cpool.tile([128, HID], bf, tag="w1")
            w1dsb = cpool.tile([128, HID], bf, tag="w1d")
            w2sb = cpool.tile([HID, HID], bf, tag="w2")
            w3sb = cpool.tile([128, 8, 113], bf, tag="w3")
            b1sb = cpool.tile([128, 1], f32, tag="b1")
            b2sb = cpool.tile([128, 1], f32, tag="b2")
            nc.gpsimd.dma_start(w1sb[:], w1p[:])
            nc.gpsimd.dma_start(w1dsb[:], w1d[:])
            nc.gpsimd.dma_start(w2sb[:], w2[:])
            nc.gpsimd.dma_start(w3sb[:], w3sp[:])
            nc.gpsimd.dma_start(b1sb[:], b1[:])
            nc.gpsimd.dma_start(b2sb[:], b2[:])

            def load_chunk(ci):
                t0 = ci * TC
                tl = min(TC, t_steps - t0)
                ftile = fpool.tile([128, TC * xw], bf, tag="f")
                for g in range(4):
                    nc.sync.dma_start(
                        ftile[32 * g:32 * g + 8, 0:tl * xw],
                        feat[g, 0:F, t0:t0 + tl, 0:xw],
                    )
                return ftile

            fcur = load_chunk(0)
            fnxt = None
            ds_prev = None
            nchunks = (t_steps + TC - 1) // TC

            for t in range(t_steps * repeats):
                u = t % t_steps
                tt = u % TC
                if tt == 0 and t > 0:
                    fcur = fnxt
                if tt == 0 and t + TC < t_steps * repeats:
                    fnxt = load_chunk((u // TC + 1) % nchunks)

                if merge_pd:
                    ds_pairs = [dpool.tile([113, NT], bf, tag="dst",
                                           name=f"ds{t}_{p}")
                                for p in range(npacks // 2)]
                else:
                    ds = dpool.tile([128, xw], bf, tag="dst")
                pd = None
                for s in range(npacks):
                    # ---- mm1: K=8 features (+ K=1 delta), 4x row-tiled;
                    #      pairs (g0,g1) and (g2,g3) share a 2-bank psum ----
                    if pair_h1:
                        pairs = [ph1pool.tile([128, 2 * NT], f32, tag="ph1",
                                              name=f"ph1p{t}_{s}_{p}")
                                 for p in range(2)]
                        ph1ap = [pairs[g // 2][:, NT * (g % 2):NT * (g % 2 + 1)]
                                 for g in range(4)]
                    else:
                        pairs = [ph1pool.tile([128, NT], f32, tag="ph1",
                                              name=f"ph1p{t}_{s}_{p}")
                                 for p in range(4)]
                        ph1ap = [pairs[g][:] for g in range(4)]
                    for g in range(4):
                        fs = tt * xw + NT * s
                        nc.tensor.matmul(
                            ph1ap[g],
                            lhsT=w1sb[32 * g:32 * g + 8, :],
                            rhs=fcur[32 * g:32 * g + 8, fs:fs + NT],
                            start=True, stop=(t == 0),
                            tile_position=(32 * g, 0),
                        )
                    if u > 0:
                        for g in range(4):
                            if merge_pd:
                                r = 32 * g + 16 * (s % 2)
                                lhsT_d = w1dsb[r:r + 1, :]
                                rhs_d = ds_prev[s // 2][r:r + 1, :]
                            else:
                                lhsT_d = w1dsb[32 * g:32 * g + 1, :]
                                rhs_d = ds_prev[32 * g:32 * g + 1, NT * s:NT * (s + 1)]
                            nc.tensor.matmul(
                                ph1ap[g],
                                lhsT=lhsT_d,
                                rhs=rhs_d,
                                start=False, stop=True,
                                tile_position=(32 * g, 0),
                            )
                    # h1 eviction: paired (one op per 2 tiles) or single
                    h1aps = []
                    if pair_h1:
                        for p in range(2):
                            h1r = h1pool.tile([128, 2 * NT], bf, tag="h1r",
                                              name=f"h1r{t}_{s}_{p}")
                            if p == 0:
                                nc.scalar.activation(h1r[:], pairs[p][:], Relu, bias=b1sb[:, 0:1])
                            else:
                                nc.vector.tensor_scalar(h1r[:], pairs[p][:], b1sb[:, 0:1], 0.0, add, amax)
                            h1aps += [h1r[:, 0:NT], h1r[:, NT:2 * NT]]
                    else:
                        for g in range(4):
                            h1r = h1pool.tile([128, NT], bf, tag="h1r",
                                              name=f"h1r{t}_{s}_{g}")
                            if g % 2 == 0:
                                nc.scalar.activation(h1r[:], pairs[g][:], Relu, bias=b1sb[:, 0:1])
                            else:
                                nc.vector.tensor_scalar(h1r[:], pairs[g][:], b1sb[:, 0:1], 0.0, add, amax)
                            h1aps.append(h1r[:])
                    if merge_pd:
                        if s % 2 == 0:
                            pd = pdpool.tile([113, NT], f32, tag="pd",
                                             name=f"pd{t}_{s}")
                        pdh = pd[:]
                    elif pair_pd:
                        if s % 2 == 0:
                            pd = pdpool.tile([97, 2 * NT], f32, tag="pd",
                                             name=f"pd{t}_{s}")
                        pdh = pd[:, NT * (s % 2):NT * (s % 2 + 1)]
                    else:
                        pd = pdpool.tile([97, NT], f32, tag="pd",
                                         name=f"pd{t}_{s}")
                        pdh = pd[:]
                    for g in range(4):
                        ph2 = ph2pool.tile([128, NT], f32, tag="ph2")
                        nc.tensor.matmul(
                            ph2[:], lhsT=w2sb[:],
                            rhs=h1aps[g],
                            start=True, stop=True)
                        h2r = h2pool.tile([128, NT], bf, tag="h2r")
                        if g % 2 == 0:
                            nc.vector.tensor_scalar(h2r[:], ph2[:], b2sb[:, 0:1], 0.0, add, amax)
                        else:
                            nc.scalar.activation(h2r[:], ph2[:], Relu, bias=b2sb[:, 0:1])
                        # delta for tile (s,g) -> pd row 32g + 16*(s%2)
                        if merge_pd:
                            nc.tensor.matmul(
                                pdh,
                                lhsT=w3sb[:, 2 * g + (s % 2), :],
                                rhs=h2r[:],
                                start=(s % 2 == 0 and g == 0),
                                stop=(s % 2 == 1 and g == 3),
                            )
                        else:
                            nc.tensor.matmul(
                                pdh,
                                lhsT=w3sb[:, 2 * g, 0:97],
                                rhs=h2r[:],
                                start=(g == 0), stop=(g == 3),
                            )
                    # delta eviction (+b3)
                    if merge_pd:
                        if s % 2 == 1:
                            p = s // 2
                            if p % 2 == 0:
                                nc.scalar.activation(ds_pairs[p][:], pd[:], Copy,
                                                     bias=float(b3_value))
                            else:
                                nc.vector.tensor_scalar(ds_pairs[p][:], pd[:],
                                                        float(b3_value), None, add)
                            nc.sync.dma_start(
                                out[u, 0:4, 1024 * p:1024 * (p + 1)],
                                ds_pairs[p][0:113:16, :],
                            )
                    elif pair_pd:
                        if s % 2 == 1:
                            dsl = ds[0:97, NT * (s - 1):NT * (s + 1)]
                            if s % 4 == 1:
                                nc.scalar.activation(dsl, pd[:], Copy, bias=float(b3_value))
                            else:
                                nc.vector.tensor_scalar(dsl, pd[:], float(b3_value), None, add)
                    else:
                        dsl = ds[0:97, NT * s:NT * (s + 1)]
                        if s % 2 == 0:
                            nc.scalar.activation(dsl, pd[:], Copy, bias=float(b3_value))
                        else:
                            nc.vector.tensor_scalar(dsl, pd[:], float(b3_value), None, add)
                if merge_pd:
                    ds_prev = ds_pairs
                else:
                    # one out-DMA for the whole step
                    nc.sync.dma_start(out[u], ds[0:97:32, :])
                    ds_prev = ds

    nc.compile()
    return nc


_NC_CACHE = {}


def _get_nc(nsh=NSH, t_steps=T_FULL, num_cores=N_CORES, b3_value=0.0):
    key = (nsh, t_steps, num_cores, float(b3_value))
    if key not in _NC_CACHE:
        _NC_CACHE[key] = build_kernel(nsh, t_steps, num_cores, b3_value)
    return _NC_CACHE[key]


def prep_core_inputs(features, W1, b1, W2, b2, W3, b3, num_cores=N_CORES):
    """Host-side shard + repack. Returns list of per-core in_maps."""
    n, t_steps, f = features.shape
    nsh = n // num_cores
    NT = 512
    npacks = nsh // (4 * NT)
    xw = npacks * NT

    w1p = np.zeros((128, HID), dtype=BF16)
    w1d = np.zeros((128, HID), dtype=BF16)
    for g in range(4):
        w1p[32 * g:32 * g + 8, :] = W1[0:8].astype(BF16)
        w1d[32 * g, :] = W1[8].astype(BF16)
        w1d[32 * g + 16, :] = W1[8].astype(BF16)
    w3sp = np.zeros((128, 8, 113), dtype=BF16)
    for g in range(4):
        for u in range(2):
            w3sp[:, 2 * g + u, 32 * g + 16 * u] = W3[:, 0].astype(BF16)
    w2b = W2.astype(BF16)
    b1c = b1.reshape(128, 1).astype(np.float32)
    b2c = b2.reshape(128, 1).astype(np.float32)

    in_maps = []
    for c in range(num_cores):
        fc = features[c * nsh:(c + 1) * nsh]          # (nsh, T, F)
        # path p = 2048s + 512g + c_ ; fpk[g, k, t, 512s + c_]
        fpk = fc.reshape(npacks, 4, NT, t_steps, f)   # (s, g, c_, t, k)
        fpk = fpk.transpose(1, 4, 3, 0, 2).reshape(4, f, t_steps, xw)
        in_maps.append({
            "features": np.ascontiguousarray(fpk).astype(BF16),
            "W1p": w1p, "W1d": w1d, "W2": w2b, "W3sp": w3sp,
            "b1": b1c, "b2": b2c,
        })
    return in_maps


def gather_out(res_core, nsh, t_steps):
    """(T, 4, xw) bf16 -> (nsh, T) fp32, path p = 2048s + 512g + c."""
    npacks = nsh // 2048
    o = np.asarray(res_core).astype(np.float32)       # (T, 4, xw)
    o = o.reshape(t_steps, 4, npacks, 512)            # (t, g, s, c)
    o = o.transpose(2, 1, 3, 0).reshape(nsh, t_steps)
    return o


def run(features, W1, b1, W2, b2, W3, b3, **run_kwargs):
    """Run on the 8 cores; returns (full_output, BassKernelResults)."""
    from concourse.bass_utils import run_bass_kernel_spmd

    features = np.asarray(features)
    n, t_steps, f = features.shape
    nsh = n // N_CORES
    in_maps = prep_core_inputs(features, W1, b1, W2, b2, W3, b3)
    nc = _get_nc(nsh, t_steps, N_CORES, float(np.asarray(b3).reshape(-1)[0]))
    res = run_bass_kernel_spmd(nc, in_maps, core_ids=list(range(N_CORES)), **run_kwargs)
    outs = [gather_out(res.results[c]["out"], nsh, t_steps) for c in range(N_CORES)]
    return np.concatenate(outs, axis=0), res


def kernel(features, W1, b1, W2, b2, W3, b3):
    out, _ = run(features, W1, b1, W2, b2, W3, b3)
    return out

